# revision 2
# baseline (speedup 1.0000x reference)
"""Trainium2 Bass kernel for nn_ChiSquareLoss (histogram binning + chi-square).

Strategy (pure data parallel across 8 NeuronCores, 4 images/core):
  - Each core receives 24 "planes" of 512x512 fp32 pixels in [0,1), laid out
    as one global column stream [128, 49152] (plane-major along free dim).
  - Per plane, a 256-bin histogram factored as 16 hi x 16 lo bins with
    CUMULATIVE (is_ge) factors; a few hi rows computed on ScalarE as
    sign(idx - c) in {-1,+1} (invertible factor matrix, host solves exactly).
  - Elementwise ops run on cross-plane chunks of fch=1408 columns (vs 1024)
    to amortize fixed per-op overhead (DVE ~145 cyc/op, ScalarE ~352 cyc/op);
    matmul PSUM accumulation start/stop is keyed by global pack index, so
    chunks may span plane boundaries.
  - S[j,i] = sum_pixels him[j]*lom[i] via TensorE outer-product matmuls,
    8 pixel-columns packed per [128,128] bf16 matmul accumulated in PSUM;
    the 8 stride-8 diagonal blocks hold S.
  - Host: S = U N V^T with known invertible U, V; recover true counts N,
    RNE-tie fixups, then chi-square + mean in float64.
"""

import sys

if "/opt/trn_rl_repo" not in sys.path:
    sys.path.insert(0, "/opt/trn_rl_repo")

from contextlib import ExitStack

import numpy as np

import concourse.bacc as bacc
import concourse.bass as bass
import concourse.tile as tile
from concourse import mybir
from concourse.bass_utils import run_bass_kernel_spmd

ALU = mybir.AluOpType
ACTF = mybir.ActivationFunctionType
F32 = mybir.dt.float32
BF16 = mybir.dt.bfloat16
I16 = mybir.dt.int16

B, C, H, W = 32, 3, 512, 512
NCORES = 8
IMGS = B // NCORES            # images per core
PLANES = IMGS * C * 2         # 24 planes per core
P = 128                       # SBUF partitions
FREE = (H * W) // P           # 2048 pixel columns per plane
GCOLS = PLANES * FREE         # 49152 global pixel columns per core
PACK = 8                      # pixel columns packed per matmul
PPP = FREE // PACK            # packs per plane = 256
NBINS = 256
BIAS = 1e-10

FCH = 1408                    # free-dim chunk size (cross-plane)
N_ACT_HI = 8                  # him rows on ScalarE via one-pass Sign

_cache = {}


def build_kernel(fch=FCH, n_act=N_ACT_HI):
    nc = bacc.Bacc()
    x_in = nc.declare_dram_parameter("x", [P, GCOLS], F32, isOutput=False)
    h_out = nc.declare_dram_parameter("h", [PLANES, P, P], F32, isOutput=True)

    nchunks = (GCOLS + fch - 1) // fch
    npack = fch // PACK

    with ExitStack() as ctx:
        tc = ctx.enter_context(tile.TileContext(nc))
        const_pool = ctx.enter_context(tc.tile_pool(name="const", bufs=1))
        pix_pool = ctx.enter_context(tc.tile_pool(name="pix", bufs=2))
        idx_pool = ctx.enter_context(tc.tile_pool(name="idx", bufs=2))
        lo_pool = ctx.enter_context(tc.tile_pool(name="lo", bufs=2))
        mask_pool = ctx.enter_context(tc.tile_pool(name="mask", bufs=1))
        psum_pool = ctx.enter_context(tc.tile_pool(name="ps", bufs=8, space="PSUM"))
        out_pool = ctx.enter_context(tc.tile_pool(name="hout", bufs=4))

        sign_bias = {}
        for j in range(16 - n_act, 16):
            t = const_pool.tile([P, 1], F32, tag=f"sb{j}")
            nc.vector.memset(t, 0.5 - 16.0 * j)
            sign_bias[j] = t

        # Ping-ponged mask sets; row 0 of each factor is the all-ones
        # cumulative row, written once.
        him_ab, lom_ab = [], []
        for h in range(2):
            him_t = mask_pool.tile([P, npack, 16, PACK], BF16, tag=f"him{h}")
            lom_t = mask_pool.tile([P, 16, fch], BF16, tag=f"lom{h}")
            nc.vector.memset(him_t[:, :, 0, :], 1.0)
            nc.vector.memset(lom_t[:, 0, :], 1.0)
            him_ab.append(him_t)
            lom_ab.append(lom_t)

        def cw(k):
            return min(fch, GCOLS - k * fch)

        def prep(k):
            w = cw(k)
            x_t = pix_pool.tile([P, fch], F32, tag="x")
            nc.sync.dma_start(out=x_t[:, :w], in_=x_in[:, k * fch:k * fch + w])
            idx = idx_pool.tile([P, fch], I16, tag="idx")
            nc.vector.tensor_scalar(idx[:, :w], x_t[:, :w], 255.0, -0.5,
                                    ALU.mult, ALU.add)
            lo = lo_pool.tile([P, fch], I16, tag="lo")
            nc.vector.tensor_scalar(lo[:, :w], idx[:, :w], 15, None,
                                    ALU.bitwise_and)
            return idx, lo

        cur = prep(0)
        ps = None
        for k in range(nchunks):
            w = cw(k)
            np_k = w // PACK          # packs in this chunk
            g0 = k * npack            # global pack index of first pack
            idx, lo = cur
            idx_r = idx.rearrange("p (s t) -> p s t", t=PACK)

            him = him_ab[k % 2]
            for j in range(1, 16 - n_act):
                nc.vector.tensor_scalar(
                    him[:, :np_k, j, :], idx_r[:, :np_k], 16.0 * j - 0.5,
                    None, ALU.is_ge
                )
            for j in range(16 - n_act, 16):
                nc.scalar.activation(
                    him[:, :np_k, j, :], idx_r[:, :np_k], ACTF.Sign,
                    bias=sign_bias[j][:, 0:1], scale=1.0,
                )

            # Prefetch next chunk's idx/lo before the lom block so ScalarE
            # can start chunk k+1's Sign rows early.
            if k + 1 < nchunks:
                cur = prep(k + 1)

            lom = lom_ab[k % 2]
            for li in range(1, 16):
                nc.vector.tensor_scalar(
                    lom[:, li, :w], lo[:, :w], li - 0.5, None, ALU.is_ge
                )

            for s in range(np_k):
                g = g0 + s
                if g % PPP == 0:
                    ps = psum_pool.tile([P, P], F32, tag="ps")
                lhsT = him[:, s].rearrange("p j t -> p (j t)")
                rhs = lom[:, :, s * PACK:(s + 1) * PACK]
                nc.tensor.matmul(
                    ps,
                    lhsT,
                    rhs,
                    start=(g % PPP == 0),
                    stop=(g % PPP == PPP - 1),
                )
                if g % PPP == PPP - 1:
                    hist_sb = out_pool.tile([P, P], F32, tag="hist")
                    nc.scalar.activation(hist_sb, ps, ACTF.Copy)
                    nc.sync.dma_start(out=h_out[g // PPP], in_=hist_sb)

    nc.finalize()
    return nc


def _get_nc():
    if "nc" not in _cache:
        _cache["nc"] = build_kernel()
    return _cache["nc"]


def shard_inputs(hist1: np.ndarray, hist2: np.ndarray):
    """Per-core device inputs: core i gets images [4i, 4i+4) of both tensors,
    as one global [128, 49152] column stream (planes along the free dim)."""
    in_maps = []
    for i in range(NCORES):
        sl1 = hist1[i * IMGS:(i + 1) * IMGS]
        sl2 = hist2[i * IMGS:(i + 1) * IMGS]
        x = np.concatenate(
            [
                np.ascontiguousarray(sl1).reshape(IMGS * C, P, FREE),
                np.ascontiguousarray(sl2).reshape(IMGS * C, P, FREE),
            ],
            axis=0,
        )  # [24, 128, 2048]
        xg = np.ascontiguousarray(
            x.transpose(1, 0, 2).reshape(P, GCOLS), dtype=np.float32
        )
        in_maps.append({"x": xg})
    return in_maps


def _recovery_mats(n_act=N_ACT_HI):
    """U (him rows) and V (lom rows): S = U N V^T -> N = Uinv S Vinv^T."""
    a = np.arange(16)
    U = (a[None, :] >= np.arange(16)[:, None]).astype(np.float64)
    U[16 - n_act:, :] = 2.0 * U[16 - n_act:, :] - 1.0
    V = (a[None, :] >= np.arange(16)[:, None]).astype(np.float64)
    return np.linalg.inv(U), np.linalg.inv(V)


_UINV, _VINV = _recovery_mats()


def hist2d_from_raw(raw: np.ndarray) -> np.ndarray:
    """raw: [..., 128, 128] PSUM accumulators -> [..., 256] histograms."""
    lead = raw.shape[:-2]
    r = raw.reshape(lead + (16, PACK, 16, PACK)).astype(np.float64)
    S = np.einsum("...jtit->...ji", r)
    N = np.einsum("ja,...ab,ib->...ji", _UINV, S, _VINV)
    N = np.rint(N)
    return N.reshape(lead + (NBINS,))


def fixup_hist(hist: np.ndarray, plane_x: np.ndarray) -> None:
    """Correct RNE tie cases in-place so counts match exact floor binning."""
    z = plane_x.astype(np.float32) * np.float32(255.0)
    zf = z[z == np.floor(z)]
    if zf.size == 0:
        return
    k = zf.astype(np.int64)
    odd = k[k % 2 == 1]
    for kk, cnt in zip(*np.unique(odd, return_counts=True)):
        hist[kk - 1] -= cnt
        hist[kk] += cnt


def finish_on_host(per_core_hists: list) -> np.ndarray:
    h = np.stack(per_core_hists)  # [8, 24, 256]
    h = h.reshape(NCORES, 2, IMGS, C, NBINS)
    counts1 = h[:, 0].reshape(B, C * NBINS)
    counts2 = h[:, 1].reshape(B, C * NBINS)
    n = float(C * H * W)
    h1 = counts1 / n
    h2 = counts2 / n
    chi = np.sum((h1 - h2) ** 2 / (h1 + h2 + BIAS), axis=1)
    return np.array(np.mean(chi), dtype=np.float32)


def kernel(hist1: np.ndarray, hist2: np.ndarray) -> np.ndarray:
    hist1 = np.asarray(hist1, dtype=np.float32)
    hist2 = np.asarray(hist2, dtype=np.float32)
    nc = _get_nc()
    in_maps = shard_inputs(hist1, hist2)
    res = run_bass_kernel_spmd(nc, in_maps, list(range(NCORES)))
    per_core = []
    for i in range(NCORES):
        hists = hist2d_from_raw(res.results[i]["h"])  # [24, 256]
        planes = in_maps[i]["x"].reshape(P, PLANES, FREE).transpose(1, 0, 2)
        for pl in range(PLANES):
            fixup_hist(hists[pl], planes[pl])
        per_core.append(hists)
    return finish_on_host(per_core)


if __name__ == "__main__":
    rng = np.random.default_rng(0)
    h1 = rng.random((B, C, H, W), dtype=np.float32)
    h2 = rng.random((B, C, H, W), dtype=np.float32)
    out = kernel(h1, h2)
    print("kernel output:", out)
